# revision 34
# baseline (speedup 1.0000x reference)
"""DeltaNet cell kernel for Trainium2 (8 NeuronCores, pure data parallel).

Per (b,h) pair:  pred = W_prev @ k;  err = v - pred;
                 W_new = W_prev + beta * err outer k;  o = W_new @ q.

o is computed as  W_prev @ q + (k.q) * beta*err  (algebraically identical),
which avoids a second pass over W_new.

Sharding: batch dim 64 -> 8 shards of 8; each core handles 8*32 = 256
independent (b,h) pairs.  All compute is on-device; host only
reshapes/slices.
"""

from contextlib import ExitStack

import numpy as np

import concourse.bass as bass
import concourse.bass_isa as bass_isa
import concourse.mybir as mybir
import concourse.tile as tile
from concourse import library_config
from concourse.bass_utils import run_bass_kernel_spmd
from concourse.masks import make_identity

B, H, D = 64, 32, 128
N_CORES = 8
B_SHARD = B // N_CORES          # 8
PAIRS = B_SHARD * H             # 256 pairs per core
C = 32                          # pairs per chunk
NCHUNKS = PAIRS // C
F32 = mybir.dt.float32


# ---------------------------------------------------------------------------
# Compatibility patches for the walrus build in this container.
#
# The bundled walrus rejects (a) the raw-ISA EVENT_SEMAPHORE_RANGE_CLEAR that
# Tile's epilogue emits (ISA length mismatch) and (b) `sem == imm` waits,
# which the stock all-engine barrier uses (wait release==0).  Both are only
# reachable through TileContext plumbing, so:
#   (a) no-op gpsimd.sem_clear (the kernel is executed on a freshly loaded
#       NEFF; semaphores start at 0 and are not reused afterwards), and
#   (b) rewrite the barrier to a monotonic-counter protocol: followers inc a
#       cumulative `gather` and wait `release >= round`; the leader waits
#       `gather >= n_followers*round` and incs `release` by 1.  Only >=-waits
#       and inc-updates remain, which this walrus supports.
# ---------------------------------------------------------------------------
_PATCHED = False


def _patch_bass_for_old_walrus():
    global _PATCHED
    if _PATCHED:
        return
    _PATCHED = True

    bass.BassGpSimd.sem_clear = lambda self, sem: None

    orig = bass.Bass._multi_engine_barrier_insts

    def monotonic_barrier(self, engines):
        insts = orig(self, engines)
        if not hasattr(self, "_mono_rounds"):
            self._mono_rounds = {}
        # identify the gather/release sem ids from the instruction set
        rel_id = None
        for i in insts:
            for w in i.sync_info.on_wait if i.sync_info else []:
                if w.ant_name.endswith("_release"):
                    rel_id = w.id
        assert rel_id is not None
        rnd = self._mono_rounds.get(rel_id, 0) + 1
        self._mono_rounds[rel_id] = rnd
        n_followers = len(engines) - 1
        for i in insts:
            si = i.sync_info
            if si is None:
                continue
            waits = list(si.on_wait)
            ups = list(si.on_update)
            if waits and waits[0].ant_name.endswith("_release") and \
                    waits[0].wait_mode == "sem-eq-imm":
                # follower drain: drop the ==0 release wait, keep gather inc
                si.on_wait = []
            elif waits and waits[0].ant_name.endswith("_release"):
                # follower release wait: wait release >= round, drop the dec
                waits[0].wait_value = rnd
                si.on_update = []
            elif waits and waits[0].ant_name.endswith("_gather"):
                # leader: wait gather >= n_followers*round, drop the dec
                waits[0].wait_value = n_followers * rnd
                si.on_update = []
            elif ups and ups[0].ant_name.endswith("_release"):
                # leader release inc: += 1 instead of += n_followers
                ups[0].update_value = 1
        return insts

    bass.Bass._multi_engine_barrier_insts = monotonic_barrier


_SPLIT_SEQ = [0]


def _split_multi_waits(nc):
    """This walrus encodes at most ONE semaphore wait per instruction.  Tile
    attaches several.  Hoist all but the last wait of each instruction into
    wait-only EventSemaphore instructions placed immediately before it on the
    same engine — the sequencer stalls there instead, which is semantically
    identical (engines execute their stream in order, and every wait's
    producer is scheduled earlier, so no new deadlock is possible)."""
    import bass_rust as _br

    def fix_block(b):
        out = []
        changed = False
        for ins in b.instructions:
            si = ins.sync_info
            if si is not None and len(si.on_wait) > 1:
                waits = list(si.on_wait)
                for w in waits[:-1]:
                    _SPLIT_SEQ[0] += 1
                    es = mybir.InstEventSemaphore(
                        name=f"SW-{_SPLIT_SEQ[0]}", ins=[], outs=[]
                    )
                    es.engine = ins.engine
                    es.sync_info = _br.SyncInfo(on_wait=[w], on_update=[])
                    out.append(es)
                si.on_wait = [waits[-1]]
                changed = True
            out.append(ins)
        if changed:
            b.instructions = out

    for f in nc.m.functions:
        for b in f.blocks:
            fix_block(b)


def _assert_no_unsupported_sync(nc):
    bad = []

    def walk(b):
        for ins in b.instructions:
            si = ins.sync_info
            if si is None:
                continue
            for w in si.on_wait:
                if "eq" in (w.wait_mode or ""):
                    bad.append((ins.name, "wait", w.wait_mode))
            for u in si.on_update:
                if "dec" in (u.update_mode or ""):
                    bad.append((ins.name, "update", u.update_mode))
        for sub in getattr(b, "blocks", []) or []:
            walk(sub)

    for f in nc.m.functions:
        for b in f.blocks:
            walk(b)
    if bad:
        raise RuntimeError(f"unsupported sync forms remain: {bad[:10]}")


def _emit(ctx: ExitStack, tc: "tile.TileContext", q_d, k_d, v_d, beta_d, w_d, o_d, wn_d):
    nc = tc.nc

    const_pool = ctx.enter_context(tc.tile_pool(name="const", bufs=1))
    ident = const_pool.tile([128, 128], F32)
    make_identity(nc, ident[:])
    ones_row = const_pool.tile([1, 128], F32)   # lhsT for partition-broadcast mms
    nc.gpsimd.memset(ones_row[:], 1.0)
    ones_col = const_pool.tile([128, 1], F32)   # lhsT for partition-reduce mms
    nc.gpsimd.memset(ones_col[:], 1.0)
    BF16 = mybir.dt.bfloat16
    ones2_bf = const_pool.tile([2, 128], BF16)  # lhsT for hi+lo k broadcast
    nc.gpsimd.memset(ones2_bf[:], 1.0)

    # SBUF pools
    w_pool = ctx.enter_context(tc.tile_pool(name="w", bufs=2))
    wn_pool = ctx.enter_context(tc.tile_pool(name="wn", bufs=2))
    wt_sb_pool = ctx.enter_context(tc.tile_pool(name="wtsb", bufs=4))
    sm_pool = ctx.enter_context(tc.tile_pool(name="sm", bufs=2))
    onat_pool = ctx.enter_context(tc.tile_pool(name="onat", bufs=2))

    # PSUM pools (8 banks total: 1 + 2 + 1 + 2 + 2; o-transpose shares misc)
    wt_ps_pool = ctx.enter_context(tc.tile_pool(name="wtps", bufs=1, space="PSUM"))
    mv_ps_pool = ctx.enter_context(tc.tile_pool(name="mvps", bufs=2, space="PSUM"))
    misc_ps_pool = ctx.enter_context(tc.tile_pool(name="miscps", bufs=1, space="PSUM"))
    krep_ps_pool = ctx.enter_context(tc.tile_pool(name="krepps", bufs=2, space="PSUM"))
    qrep_ps_pool = ctx.enter_context(tc.tile_pool(name="qrepps", bufs=2, space="PSUM"))

    G = 4                      # pairs per k_rep broadcast matmul (N = G*D = 512)
    F_PE = 16                  # pairs 0..F_PE-1 use the PE matvec pipeline
    for ch in range(NCHUNKS):
        c0 = ch * C

        # ---- DMA loads -------------------------------------------------
        w_sb = w_pool.tile([128, C * D], F32)
        nc.sync.dma_start(
            w_sb[:].rearrange("p (c j) -> p c j", c=C),
            w_d[c0 : c0 + C, :, :].rearrange("c i j -> i c j"),
        )
        k_nat = sm_pool.tile([C, D], F32, tag="knat")
        nc.sync.dma_start(k_nat[:], k_d[c0 : c0 + C, :])
        q_nat = sm_pool.tile([C, D], F32, tag="qnat")
        nc.sync.dma_start(q_nat[:], q_d[c0 : c0 + C, :])
        v_nat = sm_pool.tile([C, D], F32, tag="vnat")
        nc.sync.dma_start(v_nat[:], v_d[c0 : c0 + C, :])
        beta_row = sm_pool.tile([1, C], F32, tag="beta")
        nc.sync.dma_start(beta_row[:], beta_d[0:1, c0 : c0 + C])

        # hi/lo bf16 split of k (k_hi + k_lo == k to ~2^-18 rel), reshaped to
        # a [2, C*D] row tile so one K=2 matmul against all-ones broadcasts
        # k across partitions exactly (fp32 PSUM accumulate of the two rows).
        BF16 = mybir.dt.bfloat16
        k_hi = sm_pool.tile([C, D], BF16, tag="khi")
        nc.vector.tensor_copy(k_hi[:], k_nat[:])
        k_lo = sm_pool.tile([C, D], BF16, tag="klo")
        nc.vector.tensor_sub(k_lo[:], k_nat[:], k_hi[:])
        k_hilo = sm_pool.tile([2, C * D], BF16, tag="khilo")
        nc.sync.dma_start(k_hilo[0:1, :], k_hi[:])
        nc.sync.dma_start(k_hilo[1:2, :], k_lo[:])

        # ---- transpose k,q,v chunks (PE) + copies to SBUF (ACT) -------
        misc_ps = misc_ps_pool.tile([128, 6 * C + 128], F32)
        kt_p, qt_p, vt_p = misc_ps[:, 0:C], misc_ps[:, C:2*C], misc_ps[:, 2*C:3*C]
        beta_rep = misc_ps[:, 3*C:4*C]
        kq_rep = misc_ps[:, 4*C:5*C]
        kq_row_p = misc_ps[0:1, 5*C:5*C + C]
        nc.tensor.transpose(kt_p, k_nat[:], ident[0:C, 0:C])
        nc.tensor.transpose(qt_p, q_nat[:], ident[0:C, 0:C])
        nc.tensor.transpose(vt_p, v_nat[:], ident[0:C, 0:C])
        kt_sb = sm_pool.tile([128, C], F32, tag="kt")
        nc.scalar.copy(kt_sb[:], kt_p)
        qt_sb = sm_pool.tile([128, C], F32, tag="qt")
        nc.scalar.copy(qt_sb[:], qt_p)
        vt_sb = sm_pool.tile([128, C], F32, tag="vt")
        nc.scalar.copy(vt_sb[:], vt_p)
        # interleave k/q columns so each pair's matvec is ONE matmul (rhs =
        # [k_c | q_c], contiguous 2-column slice)
        kq_t = sm_pool.tile([128, 2 * C], F32, tag="kqt")
        kq_v = kq_t[:].rearrange("p (c two) -> p two c", two=2)
        nc.scalar.copy(kq_v[:, 0, :], kt_p)
        nc.scalar.copy(kq_v[:, 1, :], qt_p)

        # beta broadcast: ones_row.T @ beta_row -> [128, C] in PSUM
        nc.tensor.matmul(beta_rep, ones_row[:], beta_row[:], start=True, stop=True)

        # hi/lo split of q as well (for DVE-pipeline matvecs)
        q_hi = sm_pool.tile([C, D], BF16, tag="qhi")
        nc.vector.tensor_copy(q_hi[:], q_nat[:])
        q_lo = sm_pool.tile([C, D], BF16, tag="qlo")
        nc.vector.tensor_sub(q_lo[:], q_nat[:], q_hi[:])
        q_hilo = sm_pool.tile([2, C * D], BF16, tag="qhilo")
        nc.sync.dma_start(q_hilo[0:1, :], q_hi[:])
        nc.sync.dma_start(q_hilo[1:2, :], q_lo[:])

        # ---- per-group: broadcast k_rep, matvecs, err/es, update ------
        # Group lifetimes keep each k_rep/q_rep PSUM tile short-lived so the
        # bufs=2 pools pipeline across groups.
        mv_ps = mv_ps_pool.tile([128, 2 * F_PE], F32)
        mv_v = mv_ps[:].rearrange("p (c two) -> p two c", two=2)
        pw_dve = sm_pool.tile([128, 2 * (C - F_PE)], F32, tag="pwdve")
        pwd_v = pw_dve[:].rearrange("p (c two) -> p two c", two=2)
        err = sm_pool.tile([128, C], F32, tag="err")
        es = sm_pool.tile([128, C], F32, tag="es")
        wn_sb = wn_pool.tile([128, C * D], F32)
        for g in range(C // G):
            gs = slice(g * G * D, (g + 1) * G * D)
            gc = slice(g * G, (g + 1) * G)
            krep_ps = krep_ps_pool.tile([128, G * D], F32)
            nc.tensor.matmul(krep_ps[:], ones2_bf[:], k_hilo[:, gs],
                             start=True, stop=True)
            is_dve = g * G >= F_PE
            if is_dve:
                qrep_ps = qrep_ps_pool.tile([128, G * D], F32)
                nc.tensor.matmul(qrep_ps[:], ones2_bf[:], q_hilo[:, gs],
                                 start=True, stop=True)
                for c in range(g * G, (g + 1) * G):
                    cs = slice(c * D, (c + 1) * D)
                    ks = slice((c - g * G) * D, (c - g * G + 1) * D)
                    i = c - F_PE
                    scratch = sm_pool.tile([128, D], F32, tag="ttrscratch")
                    nc.vector.scalar_tensor_tensor(
                        out=scratch[:], in0=w_sb[:, cs], scalar=1.0,
                        in1=krep_ps[:, ks],
                        op0=mybir.AluOpType.mult, op1=mybir.AluOpType.mult,
                        accum_out=pw_dve[:, 2 * i : 2 * i + 1],
                    )
                    scratch2 = sm_pool.tile([128, D], F32, tag="ttrscratch2")
                    nc.vector.scalar_tensor_tensor(
                        out=scratch2[:], in0=w_sb[:, cs], scalar=1.0,
                        in1=qrep_ps[:, ks],
                        op0=mybir.AluOpType.mult, op1=mybir.AluOpType.mult,
                        accum_out=pw_dve[:, 2 * i + 1 : 2 * i + 2],
                    )
                i0 = g * G - F_PE
                nc.vector.tensor_sub(err[:, gc], vt_sb[:, gc],
                                     pwd_v[:, 0, i0 : i0 + G])
            else:
                for c in range(g * G, (g + 1) * G):
                    cs = slice(c * D, (c + 1) * D)
                    wt_ps = wt_ps_pool.tile([128, 128], F32)
                    nc.tensor.transpose(wt_ps[:], w_sb[:, cs], ident[:])
                    wt_sb = wt_sb_pool.tile([128, 128], F32)
                    nc.scalar.copy(wt_sb[:], wt_ps[:])
                    nc.tensor.matmul(mv_ps[:, 2 * c : 2 * c + 2], wt_sb[:],
                                     kq_t[:, 2 * c : 2 * c + 2],
                                     start=True, stop=True)
                nc.vector.tensor_sub(err[:, gc], vt_sb[:, gc],
                                     mv_v[:, 0, gc])
            nc.vector.tensor_mul(es[:, gc], err[:, gc], beta_rep[:, gc])
            for c in range(g * G, (g + 1) * G):
                cs = slice(c * D, (c + 1) * D)
                ks = slice((c - g * G) * D, (c - g * G + 1) * D)
                nc.vector.scalar_tensor_tensor(
                    out=wn_sb[:, cs],
                    in0=krep_ps[:, ks],
                    scalar=es[:, c : c + 1],
                    in1=w_sb[:, cs],
                    op0=mybir.AluOpType.mult,
                    op1=mybir.AluOpType.add,
                )

        # ---- o assembly (chunk-level) ---------------------------------
        # kq[c] = k_c . q_c : elementwise on transposed tiles, column-sum on
        # PE (ones_col), broadcast back to all partitions (ones_row)
        ktq = sm_pool.tile([128, C], F32, tag="ktq")
        nc.vector.tensor_mul(ktq[:], kt_sb[:], qt_sb[:])
        nc.tensor.matmul(kq_row_p, ones_col[:], ktq[:], start=True, stop=True)
        kq_row = sm_pool.tile([1, C], F32, tag="kqrow")
        nc.scalar.copy(kq_row[:], kq_row_p)
        nc.tensor.matmul(kq_rep, ones_row[:], kq_row[:], start=True, stop=True)
        t1 = sm_pool.tile([128, C], F32, tag="t1")
        nc.vector.tensor_mul(t1[:], kq_rep, es[:])
        o_sb = sm_pool.tile([128, C], F32, tag="osb")
        nc.vector.tensor_add(o_sb[:, 0:F_PE], t1[:, 0:F_PE], mv_v[:, 1, :])
        nc.vector.tensor_add(o_sb[:, F_PE:C], t1[:, F_PE:C], pwd_v[:, 1, :])

        # ---- o back to natural layout + stores ------------------------
        ot_ps = misc_ps[0:C, 6 * C : 6 * C + 128]
        nc.tensor.transpose(ot_ps, o_sb[:], ident[:])
        o_nat = onat_pool.tile([C, 128], F32)
        nc.scalar.copy(o_nat[:], ot_ps)
        nc.gpsimd.dma_start(o_d[c0 : c0 + C, :], o_nat[:])
        nc.gpsimd.dma_start(
            wn_d[c0 : c0 + C, :, :].rearrange("c i j -> i c j"),
            wn_sb[:].rearrange("p (c j) -> p c j", c=C),
        )


def _build_bass() -> bass.Bass:
    _patch_bass_for_old_walrus()
    nc = bass.Bass("TRN2", target_bir_lowering=False, debug=False)
    q_d = nc.dram_tensor("q", [PAIRS, D], F32, kind="ExternalInput").ap()
    k_d = nc.dram_tensor("k", [PAIRS, D], F32, kind="ExternalInput").ap()
    v_d = nc.dram_tensor("v", [PAIRS, D], F32, kind="ExternalInput").ap()
    beta_d = nc.dram_tensor("beta", [1, PAIRS], F32, kind="ExternalInput").ap()
    w_d = nc.dram_tensor("W_prev", [PAIRS, D, D], F32, kind="ExternalInput").ap()
    o_d = nc.dram_tensor("o", [PAIRS, D], F32, kind="ExternalOutput").ap()
    wn_d = nc.dram_tensor("W_new", [PAIRS, D, D], F32, kind="ExternalOutput").ap()

    with tile.TileContext(nc) as tc:
        with ExitStack() as ctx:
            _emit(ctx, tc, q_d, k_d, v_d, beta_d, w_d, o_d, wn_d)
    _split_multi_waits(nc)
    _assert_no_unsupported_sync(nc)
    return nc


_NC_CACHE = None


def _get_nc():
    global _NC_CACHE
    if _NC_CACHE is None:
        _NC_CACHE = _build_bass()
    return _NC_CACHE


def kernel(q, k, v, beta, W_prev, **run_kwargs):
    q = np.ascontiguousarray(np.asarray(q, dtype=np.float32))
    k = np.ascontiguousarray(np.asarray(k, dtype=np.float32))
    v = np.ascontiguousarray(np.asarray(v, dtype=np.float32))
    beta = np.ascontiguousarray(np.asarray(beta, dtype=np.float32))
    W_prev = np.ascontiguousarray(np.asarray(W_prev, dtype=np.float32))

    in_maps = []
    for s in range(N_CORES):
        sl = slice(s * B_SHARD, (s + 1) * B_SHARD)
        in_maps.append({
            "q": q[sl].reshape(PAIRS, D),
            "k": k[sl].reshape(PAIRS, D),
            "v": v[sl].reshape(PAIRS, D),
            "beta": beta[sl].reshape(1, PAIRS),
            "W_prev": W_prev[sl].reshape(PAIRS, D, D),
        })

    nc = _get_nc()
    res = run_bass_kernel_spmd(nc, in_maps, core_ids=list(range(N_CORES)), **run_kwargs)
    global _LAST_RESULTS
    _LAST_RESULTS = res
    o = np.concatenate(
        [res.results[s]["o"].reshape(B_SHARD, H, D) for s in range(N_CORES)], axis=0
    )
    wn = np.concatenate(
        [res.results[s]["W_new"].reshape(B_SHARD, H, D, D) for s in range(N_CORES)],
        axis=0,
    )
    return o, wn


if __name__ == "__main__":
    # smoke-build only
    _get_nc()
    print("bass build OK")


# revision 35
# speedup vs baseline: 1.1591x; 1.1591x over previous
"""DeltaNet cell kernel for Trainium2 (8 NeuronCores, pure data parallel).

Per (b,h) pair:  pred = W_prev @ k;  err = v - pred;
                 W_new = W_prev + beta * err outer k;  o = W_new @ q.

o is computed as  W_prev @ q + (k.q) * beta*err  (algebraically identical),
which avoids a second pass over W_new.

Sharding: batch dim 64 -> 8 shards of 8; each core handles 8*32 = 256
independent (b,h) pairs.  All compute is on-device; host only
reshapes/slices.
"""

from contextlib import ExitStack

import numpy as np

import concourse.bass as bass
import concourse.bass_isa as bass_isa
import concourse.mybir as mybir
import concourse.tile as tile
from concourse import library_config
from concourse.bass_utils import run_bass_kernel_spmd
from concourse.masks import make_identity

B, H, D = 64, 32, 128
N_CORES = 8
B_SHARD = B // N_CORES          # 8
PAIRS = B_SHARD * H             # 256 pairs per core
C = 32                          # pairs per chunk
NCHUNKS = PAIRS // C
F32 = mybir.dt.float32


# ---------------------------------------------------------------------------
# Compatibility patches for the walrus build in this container.
#
# The bundled walrus rejects (a) the raw-ISA EVENT_SEMAPHORE_RANGE_CLEAR that
# Tile's epilogue emits (ISA length mismatch) and (b) `sem == imm` waits,
# which the stock all-engine barrier uses (wait release==0).  Both are only
# reachable through TileContext plumbing, so:
#   (a) no-op gpsimd.sem_clear (the kernel is executed on a freshly loaded
#       NEFF; semaphores start at 0 and are not reused afterwards), and
#   (b) rewrite the barrier to a monotonic-counter protocol: followers inc a
#       cumulative `gather` and wait `release >= round`; the leader waits
#       `gather >= n_followers*round` and incs `release` by 1.  Only >=-waits
#       and inc-updates remain, which this walrus supports.
# ---------------------------------------------------------------------------
_PATCHED = False


def _patch_bass_for_old_walrus():
    global _PATCHED
    if _PATCHED:
        return
    _PATCHED = True

    bass.BassGpSimd.sem_clear = lambda self, sem: None

    orig = bass.Bass._multi_engine_barrier_insts

    def monotonic_barrier(self, engines):
        insts = orig(self, engines)
        if not hasattr(self, "_mono_rounds"):
            self._mono_rounds = {}
        # identify the gather/release sem ids from the instruction set
        rel_id = None
        for i in insts:
            for w in i.sync_info.on_wait if i.sync_info else []:
                if w.ant_name.endswith("_release"):
                    rel_id = w.id
        assert rel_id is not None
        rnd = self._mono_rounds.get(rel_id, 0) + 1
        self._mono_rounds[rel_id] = rnd
        n_followers = len(engines) - 1
        for i in insts:
            si = i.sync_info
            if si is None:
                continue
            waits = list(si.on_wait)
            ups = list(si.on_update)
            if waits and waits[0].ant_name.endswith("_release") and \
                    waits[0].wait_mode == "sem-eq-imm":
                # follower drain: drop the ==0 release wait, keep gather inc
                si.on_wait = []
            elif waits and waits[0].ant_name.endswith("_release"):
                # follower release wait: wait release >= round, drop the dec
                waits[0].wait_value = rnd
                si.on_update = []
            elif waits and waits[0].ant_name.endswith("_gather"):
                # leader: wait gather >= n_followers*round, drop the dec
                waits[0].wait_value = n_followers * rnd
                si.on_update = []
            elif ups and ups[0].ant_name.endswith("_release"):
                # leader release inc: += 1 instead of += n_followers
                ups[0].update_value = 1
        return insts

    bass.Bass._multi_engine_barrier_insts = monotonic_barrier


_SPLIT_SEQ = [0]


def _split_multi_waits(nc):
    """This walrus encodes at most ONE semaphore wait per instruction.  Tile
    attaches several.  Hoist all but the last wait of each instruction into
    wait-only EventSemaphore instructions placed immediately before it on the
    same engine — the sequencer stalls there instead, which is semantically
    identical (engines execute their stream in order, and every wait's
    producer is scheduled earlier, so no new deadlock is possible)."""
    import bass_rust as _br

    def fix_block(b):
        out = []
        changed = False
        for ins in b.instructions:
            si = ins.sync_info
            if si is not None and len(si.on_wait) > 1:
                waits = list(si.on_wait)
                for w in waits[:-1]:
                    _SPLIT_SEQ[0] += 1
                    es = mybir.InstEventSemaphore(
                        name=f"SW-{_SPLIT_SEQ[0]}", ins=[], outs=[]
                    )
                    es.engine = ins.engine
                    es.sync_info = _br.SyncInfo(on_wait=[w], on_update=[])
                    out.append(es)
                si.on_wait = [waits[-1]]
                changed = True
            out.append(ins)
        if changed:
            b.instructions = out

    for f in nc.m.functions:
        for b in f.blocks:
            fix_block(b)


def _assert_no_unsupported_sync(nc):
    bad = []

    def walk(b):
        for ins in b.instructions:
            si = ins.sync_info
            if si is None:
                continue
            for w in si.on_wait:
                if "eq" in (w.wait_mode or ""):
                    bad.append((ins.name, "wait", w.wait_mode))
            for u in si.on_update:
                if "dec" in (u.update_mode or ""):
                    bad.append((ins.name, "update", u.update_mode))
        for sub in getattr(b, "blocks", []) or []:
            walk(sub)

    for f in nc.m.functions:
        for b in f.blocks:
            walk(b)
    if bad:
        raise RuntimeError(f"unsupported sync forms remain: {bad[:10]}")


def _emit(ctx: ExitStack, tc: "tile.TileContext", q_d, k_d, v_d, beta_d, w_d, o_d, wn_d):
    nc = tc.nc

    const_pool = ctx.enter_context(tc.tile_pool(name="const", bufs=1))
    ident = const_pool.tile([128, 128], F32)
    make_identity(nc, ident[:])
    ones_row = const_pool.tile([1, 128], F32)   # lhsT for partition-broadcast mms
    nc.gpsimd.memset(ones_row[:], 1.0)
    ones_col = const_pool.tile([128, 1], F32)   # lhsT for partition-reduce mms
    nc.gpsimd.memset(ones_col[:], 1.0)
    BF16 = mybir.dt.bfloat16
    ones2_bf = const_pool.tile([2, 128], BF16)  # lhsT for hi+lo k broadcast
    nc.gpsimd.memset(ones2_bf[:], 1.0)

    # SBUF pools
    w_pool = ctx.enter_context(tc.tile_pool(name="w", bufs=2))
    wn_pool = ctx.enter_context(tc.tile_pool(name="wn", bufs=2))
    wt_sb_pool = ctx.enter_context(tc.tile_pool(name="wtsb", bufs=4))
    sm_pool = ctx.enter_context(tc.tile_pool(name="sm", bufs=2))
    onat_pool = ctx.enter_context(tc.tile_pool(name="onat", bufs=2))

    # PSUM pools (8 banks total: 1 + 2 + 1 + 2 + 2; o-transpose shares misc)
    wt_ps_pool = ctx.enter_context(tc.tile_pool(name="wtps", bufs=2, space="PSUM"))
    mv_ps_pool = ctx.enter_context(tc.tile_pool(name="mvps", bufs=2, space="PSUM"))
    misc_ps_pool = ctx.enter_context(tc.tile_pool(name="miscps", bufs=1, space="PSUM"))
    krep_ps_pool = ctx.enter_context(tc.tile_pool(name="krepps", bufs=2, space="PSUM"))
    qrep_ps_pool = ctx.enter_context(tc.tile_pool(name="qrepps", bufs=1, space="PSUM"))

    G = 4                      # pairs per k_rep broadcast matmul (N = G*D = 512)
    F_PE = 16                  # pairs 0..F_PE-1 use the PE matvec pipeline
    for ch in range(NCHUNKS):
        c0 = ch * C

        # ---- DMA loads -------------------------------------------------
        w_sb = w_pool.tile([128, C * D], F32)
        nc.sync.dma_start(
            w_sb[:].rearrange("p (c j) -> p c j", c=C),
            w_d[c0 : c0 + C, :, :].rearrange("c i j -> i c j"),
        )
        k_nat = sm_pool.tile([C, D], F32, tag="knat")
        nc.sync.dma_start(k_nat[:], k_d[c0 : c0 + C, :])
        q_nat = sm_pool.tile([C, D], F32, tag="qnat")
        nc.sync.dma_start(q_nat[:], q_d[c0 : c0 + C, :])
        v_nat = sm_pool.tile([C, D], F32, tag="vnat")
        nc.sync.dma_start(v_nat[:], v_d[c0 : c0 + C, :])
        beta_row = sm_pool.tile([1, C], F32, tag="beta")
        nc.sync.dma_start(beta_row[:], beta_d[0:1, c0 : c0 + C])

        # hi/lo bf16 split of k (k_hi + k_lo == k to ~2^-18 rel), reshaped to
        # a [2, C*D] row tile so one K=2 matmul against all-ones broadcasts
        # k across partitions exactly (fp32 PSUM accumulate of the two rows).
        BF16 = mybir.dt.bfloat16
        k_hi = sm_pool.tile([C, D], BF16, tag="khi")
        nc.vector.tensor_copy(k_hi[:], k_nat[:])
        k_lo = sm_pool.tile([C, D], BF16, tag="klo")
        nc.vector.tensor_sub(k_lo[:], k_nat[:], k_hi[:])
        k_hilo = sm_pool.tile([2, C * D], BF16, tag="khilo")
        nc.sync.dma_start(k_hilo[0:1, :], k_hi[:])
        nc.sync.dma_start(k_hilo[1:2, :], k_lo[:])

        # ---- transpose k,q,v chunks (PE) + copies to SBUF (ACT) -------
        misc_ps = misc_ps_pool.tile([128, 6 * C + 128], F32)
        kt_p, qt_p, vt_p = misc_ps[:, 0:C], misc_ps[:, C:2*C], misc_ps[:, 2*C:3*C]
        beta_rep = misc_ps[:, 3*C:4*C]
        kq_rep = misc_ps[:, 4*C:5*C]
        kq_row_p = misc_ps[0:1, 5*C:5*C + C]
        nc.tensor.transpose(kt_p, k_nat[:], ident[0:C, 0:C])
        nc.tensor.transpose(qt_p, q_nat[:], ident[0:C, 0:C])
        nc.tensor.transpose(vt_p, v_nat[:], ident[0:C, 0:C])
        kt_sb = sm_pool.tile([128, C], F32, tag="kt")
        nc.scalar.copy(kt_sb[:], kt_p)
        qt_sb = sm_pool.tile([128, C], F32, tag="qt")
        nc.scalar.copy(qt_sb[:], qt_p)
        vt_sb = sm_pool.tile([128, C], F32, tag="vt")
        nc.scalar.copy(vt_sb[:], vt_p)
        # interleave k/q columns so each pair's matvec is ONE matmul (rhs =
        # [k_c | q_c], contiguous 2-column slice)
        kq_t = sm_pool.tile([128, 2 * C], F32, tag="kqt")
        kq_v = kq_t[:].rearrange("p (c two) -> p two c", two=2)
        nc.scalar.copy(kq_v[:, 0, :], kt_p)
        nc.scalar.copy(kq_v[:, 1, :], qt_p)

        # beta broadcast: ones_row.T @ beta_row -> [128, C] in PSUM
        nc.tensor.matmul(beta_rep, ones_row[:], beta_row[:], start=True, stop=True)

        # hi/lo split of q as well (for DVE-pipeline matvecs)
        q_hi = sm_pool.tile([C, D], BF16, tag="qhi")
        nc.vector.tensor_copy(q_hi[:], q_nat[:])
        q_lo = sm_pool.tile([C, D], BF16, tag="qlo")
        nc.vector.tensor_sub(q_lo[:], q_nat[:], q_hi[:])
        q_hilo = sm_pool.tile([2, C * D], BF16, tag="qhilo")
        nc.sync.dma_start(q_hilo[0:1, :], q_hi[:])
        nc.sync.dma_start(q_hilo[1:2, :], q_lo[:])

        # ---- per-group: broadcast k_rep, matvecs, err/es, update ------
        # Group lifetimes keep each k_rep/q_rep PSUM tile short-lived so the
        # bufs=2 pools pipeline across groups.
        mv_ps = mv_ps_pool.tile([128, 2 * F_PE], F32)
        mv_v = mv_ps[:].rearrange("p (c two) -> p two c", two=2)
        pw_dve = sm_pool.tile([128, 2 * (C - F_PE)], F32, tag="pwdve")
        pwd_v = pw_dve[:].rearrange("p (c two) -> p two c", two=2)
        err = sm_pool.tile([128, C], F32, tag="err")
        es = sm_pool.tile([128, C], F32, tag="es")
        wn_sb = wn_pool.tile([128, C * D], F32)
        for g in range(C // G):
            gs = slice(g * G * D, (g + 1) * G * D)
            gc = slice(g * G, (g + 1) * G)
            krep_ps = krep_ps_pool.tile([128, G * D], F32)
            nc.tensor.matmul(krep_ps[:], ones2_bf[:], k_hilo[:, gs],
                             start=True, stop=True)
            is_dve = g * G >= F_PE
            if is_dve:
                qrep_ps = qrep_ps_pool.tile([128, G * D], F32)
                nc.tensor.matmul(qrep_ps[:], ones2_bf[:], q_hilo[:, gs],
                                 start=True, stop=True)
                for c in range(g * G, (g + 1) * G):
                    cs = slice(c * D, (c + 1) * D)
                    ks = slice((c - g * G) * D, (c - g * G + 1) * D)
                    i = c - F_PE
                    scratch = sm_pool.tile([128, D], F32, tag="ttrscratch")
                    nc.vector.scalar_tensor_tensor(
                        out=scratch[:], in0=w_sb[:, cs], scalar=1.0,
                        in1=krep_ps[:, ks],
                        op0=mybir.AluOpType.mult, op1=mybir.AluOpType.mult,
                        accum_out=pw_dve[:, 2 * i : 2 * i + 1],
                    )
                    scratch2 = sm_pool.tile([128, D], F32, tag="ttrscratch2")
                    nc.vector.scalar_tensor_tensor(
                        out=scratch2[:], in0=w_sb[:, cs], scalar=1.0,
                        in1=qrep_ps[:, ks],
                        op0=mybir.AluOpType.mult, op1=mybir.AluOpType.mult,
                        accum_out=pw_dve[:, 2 * i + 1 : 2 * i + 2],
                    )
                i0 = g * G - F_PE
                nc.vector.tensor_sub(err[:, gc], vt_sb[:, gc],
                                     pwd_v[:, 0, i0 : i0 + G])
            else:
                for c in range(g * G, (g + 1) * G):
                    cs = slice(c * D, (c + 1) * D)
                    wt_ps = wt_ps_pool.tile([128, 128], F32)
                    nc.tensor.transpose(wt_ps[:], w_sb[:, cs], ident[:])
                    wt_sb = wt_sb_pool.tile([128, 128], F32)
                    nc.scalar.copy(wt_sb[:], wt_ps[:])
                    nc.tensor.matmul(mv_ps[:, 2 * c : 2 * c + 2], wt_sb[:],
                                     kq_t[:, 2 * c : 2 * c + 2],
                                     start=True, stop=True)
                nc.vector.tensor_sub(err[:, gc], vt_sb[:, gc],
                                     mv_v[:, 0, gc])
            nc.vector.tensor_mul(es[:, gc], err[:, gc], beta_rep[:, gc])
            for c in range(g * G, (g + 1) * G):
                cs = slice(c * D, (c + 1) * D)
                ks = slice((c - g * G) * D, (c - g * G + 1) * D)
                nc.vector.scalar_tensor_tensor(
                    out=wn_sb[:, cs],
                    in0=krep_ps[:, ks],
                    scalar=es[:, c : c + 1],
                    in1=w_sb[:, cs],
                    op0=mybir.AluOpType.mult,
                    op1=mybir.AluOpType.add,
                )

        # ---- o assembly (chunk-level) ---------------------------------
        # kq[c] = k_c . q_c : elementwise on transposed tiles, column-sum on
        # PE (ones_col), broadcast back to all partitions (ones_row)
        ktq = sm_pool.tile([128, C], F32, tag="ktq")
        nc.vector.tensor_mul(ktq[:], kt_sb[:], qt_sb[:])
        nc.tensor.matmul(kq_row_p, ones_col[:], ktq[:], start=True, stop=True)
        kq_row = sm_pool.tile([1, C], F32, tag="kqrow")
        nc.scalar.copy(kq_row[:], kq_row_p)
        nc.tensor.matmul(kq_rep, ones_row[:], kq_row[:], start=True, stop=True)
        t1 = sm_pool.tile([128, C], F32, tag="t1")
        nc.vector.tensor_mul(t1[:], kq_rep, es[:])
        o_sb = sm_pool.tile([128, C], F32, tag="osb")
        nc.vector.tensor_add(o_sb[:, 0:F_PE], t1[:, 0:F_PE], mv_v[:, 1, :])
        nc.vector.tensor_add(o_sb[:, F_PE:C], t1[:, F_PE:C], pwd_v[:, 1, :])

        # ---- o back to natural layout + stores ------------------------
        ot_ps = misc_ps[0:C, 6 * C : 6 * C + 128]
        nc.tensor.transpose(ot_ps, o_sb[:], ident[:])
        o_nat = onat_pool.tile([C, 128], F32)
        nc.scalar.copy(o_nat[:], ot_ps)
        nc.gpsimd.dma_start(o_d[c0 : c0 + C, :], o_nat[:])
        nc.gpsimd.dma_start(
            wn_d[c0 : c0 + C, :, :].rearrange("c i j -> i c j"),
            wn_sb[:].rearrange("p (c j) -> p c j", c=C),
        )


def _build_bass() -> bass.Bass:
    _patch_bass_for_old_walrus()
    nc = bass.Bass("TRN2", target_bir_lowering=False, debug=False)
    q_d = nc.dram_tensor("q", [PAIRS, D], F32, kind="ExternalInput").ap()
    k_d = nc.dram_tensor("k", [PAIRS, D], F32, kind="ExternalInput").ap()
    v_d = nc.dram_tensor("v", [PAIRS, D], F32, kind="ExternalInput").ap()
    beta_d = nc.dram_tensor("beta", [1, PAIRS], F32, kind="ExternalInput").ap()
    w_d = nc.dram_tensor("W_prev", [PAIRS, D, D], F32, kind="ExternalInput").ap()
    o_d = nc.dram_tensor("o", [PAIRS, D], F32, kind="ExternalOutput").ap()
    wn_d = nc.dram_tensor("W_new", [PAIRS, D, D], F32, kind="ExternalOutput").ap()

    with tile.TileContext(nc) as tc:
        with ExitStack() as ctx:
            _emit(ctx, tc, q_d, k_d, v_d, beta_d, w_d, o_d, wn_d)
    _split_multi_waits(nc)
    _assert_no_unsupported_sync(nc)
    return nc


_NC_CACHE = None


def _get_nc():
    global _NC_CACHE
    if _NC_CACHE is None:
        _NC_CACHE = _build_bass()
    return _NC_CACHE


def kernel(q, k, v, beta, W_prev, **run_kwargs):
    q = np.ascontiguousarray(np.asarray(q, dtype=np.float32))
    k = np.ascontiguousarray(np.asarray(k, dtype=np.float32))
    v = np.ascontiguousarray(np.asarray(v, dtype=np.float32))
    beta = np.ascontiguousarray(np.asarray(beta, dtype=np.float32))
    W_prev = np.ascontiguousarray(np.asarray(W_prev, dtype=np.float32))

    in_maps = []
    for s in range(N_CORES):
        sl = slice(s * B_SHARD, (s + 1) * B_SHARD)
        in_maps.append({
            "q": q[sl].reshape(PAIRS, D),
            "k": k[sl].reshape(PAIRS, D),
            "v": v[sl].reshape(PAIRS, D),
            "beta": beta[sl].reshape(1, PAIRS),
            "W_prev": W_prev[sl].reshape(PAIRS, D, D),
        })

    nc = _get_nc()
    res = run_bass_kernel_spmd(nc, in_maps, core_ids=list(range(N_CORES)), **run_kwargs)
    global _LAST_RESULTS
    _LAST_RESULTS = res
    o = np.concatenate(
        [res.results[s]["o"].reshape(B_SHARD, H, D) for s in range(N_CORES)], axis=0
    )
    wn = np.concatenate(
        [res.results[s]["W_new"].reshape(B_SHARD, H, D, D) for s in range(N_CORES)],
        axis=0,
    )
    return o, wn


if __name__ == "__main__":
    # smoke-build only
    _get_nc()
    print("bass build OK")
